# revision 1
# baseline (speedup 1.0000x reference)
"""BERT-LSTM-CRF kernel for Trainium2, 8 NeuronCores.

Sharding: direction x batch split. Cores 0-3: forward LSTM over batch
quarters (16 samples each); cores 4-7: backward LSTM over the same
quarters (fed time-reversed embeddings so the same SPMD program runs
everywhere). Each core: input GEMM (embeds @ W_ih^T + bias via an
augmented ones-column), 512-step LSTM scan (PE recurrent matmul in
fp32r, moving operand = W_hh^T streamed at 1 cyc/row), and the output
projection of its half of the hidden concat. Host does the (pure data
movement) wordpiece gather / alignment, time reversal for the backward
cores, and the final fwd+bwd partial-sum + bias add.
"""
import os
import sys
import numpy as np

sys.path.insert(0, "/opt/trn_rl_repo")

B, S, D, H, T = 64, 512, 768, 384, 22
G4 = 4 * H            # 1536 gate rows
DA = 896              # 768 + 1 ones-column + zero pad to 7*128
BL = 16               # batch per core
NC = 8
STEPS = int(os.environ.get("KSTEPS", str(S)))
MM_DTYPE = os.environ.get("KMMDT", "float32r")  # float32r | float32
XG_CHUNK = 2          # scan timesteps per xg DMA chunk

_cache = {}


def _align_np(hidden_states, start_ids, masks):
    """numpy port of reference._align."""
    hs = np.asarray(hidden_states)
    sid = np.asarray(start_ids)
    msk = np.asarray(masks)
    Bb, Ss, _ = hs.shape
    t = np.arange(Ss)[None, :]
    valid = sid >= 0
    n = valid.sum(-1)
    last_sid = np.take_along_axis(sid, (n - 1)[:, None], axis=1)
    idx = np.where(t == 0, 0,
          np.where(t < n[:, None], sid - 1,
          np.where(t == n[:, None], last_sid, 0)))
    idx = np.clip(idx, 0, Ss - 1).astype(np.int64)
    gathered = np.take_along_axis(hs, idx[:, :, None], axis=1)
    sent_len = msk.sum(-1)
    keep = (t < sent_len[:, None])[:, :, None]
    return np.where(keep, gathered, 0.0).astype(np.float32)


def _build_program():
    from concourse import bass, bacc, tile, mybir
    from contextlib import ExitStack

    f32 = mybir.dt.float32
    mmdt = getattr(mybir.dt, MM_DTYPE)
    AF = mybir.ActivationFunctionType

    nc = bacc.Bacc("TRN2", target_bir_lowering=False, debug=False,
                   num_devices=NC)

    emb = nc.dram_tensor("emb", [BL * S, DA], f32, kind="ExternalInput")
    wih = nc.dram_tensor("wih", [DA, G4], mmdt, kind="ExternalInput")
    whh = nc.dram_tensor("whh", [H, G4], mmdt, kind="ExternalInput")
    wlin = nc.dram_tensor("wlin", [H, T], mmdt, kind="ExternalInput")
    h0t = nc.dram_tensor("h0t", [H, BL], mmdt, kind="ExternalInput")
    c0 = nc.dram_tensor("c0", [BL, H], f32, kind="ExternalInput")
    id16 = nc.dram_tensor("id16", [16, 16], f32, kind="ExternalInput")
    id128 = nc.dram_tensor("id128", [128, 128], f32, kind="ExternalInput")
    partial = nc.dram_tensor("partial", [S * BL, T], f32,
                             kind="ExternalOutput")
    xg_dram = nc.dram_tensor("xg_scratch", [S, BL, G4], f32)

    RT = (BL * S) // 128  # 64 row tiles of the input GEMM
    KD = DA // 128        # 7 contraction chunks (incl. bias/pad)
    KH = H // 128         # 3 hidden chunks

    with tile.TileContext(nc) as tc, ExitStack() as big:
        # --- persistent SBUF tiles ---
        consts = big.enter_context(tc.tile_pool(name="consts", bufs=1))
        hist_pool = big.enter_context(tc.tile_pool(name="hist", bufs=1))

        id16_sb = consts.tile([16, 16], f32, tag="id16")
        nc.sync.dma_start(id16_sb[:], id16[:])
        id128_sb = consts.tile([128, 128], f32, tag="id128")
        nc.sync.dma_start(id128_sb[:], id128[:])
        whh_all = consts.tile([128, KH, G4], mmdt, tag="whh")
        nc.sync.dma_start(whh_all[:],
                          whh.rearrange("(k p) g -> p k g", p=128))
        whh_sb = [whh_all[:, k, :] for k in range(KH)]
        h0t_all = consts.tile([128, KH, BL], mmdt, tag="h0t")
        nc.sync.dma_start(h0t_all[:],
                          h0t.rearrange("(k p) b -> p k b", p=128))
        h0t_sb = [h0t_all[:, k, :] for k in range(KH)]
        c0_sb = consts.tile([BL, H], f32, tag="c0")
        nc.sync.dma_start(c0_sb[:], c0[:])
        wlin_all = consts.tile([128, KH, T], mmdt, tag="wlin")
        nc.sync.dma_start(wlin_all[:],
                          wlin.rearrange("(k p) t -> p k t", p=128))
        wlin_sb = [wlin_all[:, k, :] for k in range(KH)]

        # hidden history, transposed: hist[k][128, BL*S], col = t*BL + b
        hist = []
        for k in range(KH):
            hist_t = hist_pool.tile([128, BL * S], mmdt, tag=f"hist{k}")
            hist.append(hist_t)

        # ---------- phase 1: xg = emb_aug @ wih (bias via ones col) ----
        with ExitStack() as ph1:
            wp = ph1.enter_context(tc.tile_pool(name="wih", bufs=1))
            wih_all = wp.tile([128, KD, G4], mmdt, tag="wih")
            nc.sync.dma_start(wih_all[:],
                              wih.rearrange("(k p) g -> p k g", p=128))
            wih_sb = [wih_all[:, k, :] for k in range(KD)]
            ep = ph1.enter_context(tc.tile_pool(name="emb", bufs=3))
            etp = ph1.enter_context(tc.tile_pool(name="embT", bufs=3))
            pp = ph1.enter_context(
                tc.tile_pool(name="ph1ps", bufs=2, space="PSUM"))
            xp = ph1.enter_context(
                tc.tile_pool(name="xgps", bufs=3, space="PSUM"))
            for rt in range(RT):
                emb_sb = ep.tile([128, DA], f32, tag="emb")
                nc.sync.dma_start(emb_sb[:], emb[rt * 128:(rt + 1) * 128, :])
                etps = pp.tile([128, 512], f32, tag="etps")
                etsb = etp.tile([128, DA], mmdt, tag="etsb")
                for k in range(KD):
                    ps = etps[:, (k % 4) * 128:(k % 4) * 128 + 128]
                    nc.tensor.transpose(ps, emb_sb[:, k * 128:(k + 1) * 128],
                                        id128_sb[:])
                    nc.vector.tensor_copy(etsb[:, k * 128:(k + 1) * 128], ps)
                b_idx, tq = rt // 4, rt % 4
                for n in range(3):
                    xps = xp.tile([128, 512], f32, tag="xps")
                    for k in range(KD):
                        nc.tensor.matmul(
                            xps[:],
                            etsb[:, k * 128:(k + 1) * 128],
                            wih_sb[k][:, n * 512:(n + 1) * 512],
                            start=(k == 0), stop=(k == KD - 1))
                    xsb = etp.tile([128, 512], f32, tag="xsb")
                    nc.vector.tensor_copy(xsb[:], xps[:])
                    # rows of this tile are t = tq*128 .. tq*128+127, one b
                    dst = xg_dram[tq * 128:(tq + 1) * 128, b_idx, n * 512:(n + 1) * 512]
                    nc.sync.dma_start(dst, xsb[:])

        # ---------- phase 2: LSTM scan ----------
        with ExitStack() as ph2:
            xgp = ph2.enter_context(tc.tile_pool(name="xgin", bufs=2))
            gp = ph2.enter_context(
                tc.tile_pool(name="gps", bufs=2, space="PSUM"))
            htp = ph2.enter_context(
                tc.tile_pool(name="htps", bufs=2, space="PSUM"))
            sp = ph2.enter_context(tc.tile_pool(name="scan", bufs=2))
            cp = ph2.enter_context(tc.tile_pool(name="cbuf", bufs=2))

            c_prev = c0_sb
            xg_sb = None
            for t in range(STEPS):
                tl = t % XG_CHUNK
                if tl == 0:
                    xg_sb = xgp.tile([BL, XG_CHUNK, G4], f32, tag="xg")
                    nc.sync.dma_start(
                        xg_sb[:],
                        xg_dram[t:t + XG_CHUNK, :, :].rearrange(
                            "t b g -> b t g"))
                if t == 0:
                    hT = [h0t_sb[k][:, :] for k in range(KH)]
                else:
                    hT = [hist[k][:, (t - 1) * BL:t * BL] for k in range(KH)]

                g_ps = gp.tile([BL, G4], f32, tag="g")
                for n in range(3):
                    for k in range(KH):
                        nc.tensor.matmul(
                            g_ps[:, n * 512:(n + 1) * 512],
                            hT[k],
                            whh_sb[k][:, n * 512:(n + 1) * 512],
                            start=(k == 0), stop=(k == KH - 1))
                g_sb = sp.tile([BL, G4], f32, tag="gsb")
                for n in range(3):
                    sl = slice(n * 512, (n + 1) * 512)
                    nc.vector.tensor_add(
                        g_sb[:, sl], g_ps[:, sl],
                        xg_sb[:, tl, n * 512:(n + 1) * 512])
                # gate order in weights was permuted to [i, f, o, g]
                a_sb = sp.tile([BL, G4], f32, tag="asb")
                nc.scalar.activation(a_sb[:, 0:3 * H], g_sb[:, 0:3 * H],
                                     AF.Sigmoid)
                nc.scalar.activation(a_sb[:, 3 * H:G4], g_sb[:, 3 * H:G4],
                                     AF.Tanh)
                c_new = cp.tile([BL, H], f32, tag="c")
                tmp = sp.tile([BL, 2 * H], f32, tag="tmp")
                nc.vector.tensor_mul(tmp[:, 0:H], a_sb[:, 0:H],
                                     a_sb[:, 3 * H:G4])          # i*tanh(g)
                nc.vector.tensor_mul(c_new[:], a_sb[:, H:2 * H], c_prev[:])
                nc.vector.tensor_add(c_new[:], c_new[:], tmp[:, 0:H])
                nc.scalar.activation(tmp[:, H:2 * H], c_new[:], AF.Tanh)
                h_sb = sp.tile([BL, H], f32, tag="h")
                nc.vector.tensor_mul(h_sb[:], a_sb[:, 2 * H:3 * H],
                                     tmp[:, H:2 * H])
                ht_ps = htp.tile([128, KH * BL], f32, tag="htps")
                for k in range(KH):
                    nc.tensor.transpose(ht_ps[:, k * BL:(k + 1) * BL],
                                        h_sb[:, k * 128:(k + 1) * 128],
                                        id16_sb[:])
                for k in range(KH):
                    nc.vector.tensor_copy(hist[k][:, t * BL:(t + 1) * BL],
                                          ht_ps[:, k * BL:(k + 1) * BL])
                c_prev = c_new

        # ---------- phase 3: feats partial = hist^T @ wlin ----------
        with ExitStack() as ph3:
            fp = ph3.enter_context(
                tc.tile_pool(name="fps", bufs=2, space="PSUM"))
            fsb = ph3.enter_context(tc.tile_pool(name="fsb", bufs=2))
            NCH = (BL * S) // 512  # 16 chunks of 512 (t,b) columns
            for j in range(NCH):
                f_ps = fp.tile([T, 512], f32, tag="f")
                for k in range(KH):
                    nc.tensor.matmul(
                        f_ps[:],
                        wlin_sb[k],
                        hist[k][:, j * 512:(j + 1) * 512],
                        start=(k == 0), stop=(k == KH - 1))
                f_sb = fsb.tile([T, 512], f32, tag="fsb")
                nc.vector.tensor_copy(f_sb[:], f_ps[:])
                nc.sync.dma_start(
                    partial[j * 512:(j + 1) * 512, :].rearrange("r t -> t r"),
                    f_sb[:])

    nc.compile()
    return nc


def _get_program():
    if "nc" not in _cache:
        _cache["nc"] = _build_program()
    return _cache["nc"]


# gate-order permutation: torch [i,f,g,o] -> kernel [i,f,o,g]
_PERM = np.concatenate([np.arange(0, H), np.arange(H, 2 * H),
                        np.arange(3 * H, 4 * H), np.arange(2 * H, 3 * H)])


def _prep_core_inputs(embeds, h0, c0, W_ih, W_hh, b_ih, b_hh, W_lin, reverse):
    """Build the per-core input map. embeds: [BL, S, D] already aligned."""
    e = embeds
    if reverse:
        e = e[:, ::-1, :]
    ea = np.zeros((BL, S, DA), np.float32)
    ea[:, :, :D] = e
    ea[:, :, D] = 1.0
    wih_a = np.zeros((DA, G4), np.float32)
    wih_a[:D, :] = W_ih.T[:, _PERM]
    wih_a[D, :] = (b_ih + b_hh)[_PERM]
    whh_t = np.ascontiguousarray(W_hh.T[:, _PERM], np.float32)
    half = slice(0, H) if not reverse else slice(H, 2 * H)
    wlin_t = np.ascontiguousarray(W_lin[:, half].T, np.float32)
    return {
        "emb": ea.reshape(BL * S, DA),
        "wih": wih_a,
        "whh": whh_t,
        "wlin": wlin_t,
        "h0t": np.ascontiguousarray(h0.T, np.float32),
        "c0": np.ascontiguousarray(c0, np.float32),
        "id16": np.eye(16, dtype=np.float32),
        "id128": np.eye(128, dtype=np.float32),
    }


def kernel(hidden_states, h0, c0, W_ih_f, W_hh_f, b_ih_f, b_hh_f,
           W_ih_b, W_hh_b, b_ih_b, b_hh_b, W_lin, b_lin, start_ids, masks,
           _trace=False):
    from concourse.bass_utils import run_bass_kernel_spmd

    hidden_states = np.asarray(hidden_states, np.float32)
    h0 = np.asarray(h0, np.float32)
    c0 = np.asarray(c0, np.float32)

    embeds = _align_np(hidden_states, start_ids, masks)

    in_maps = []
    for core in range(NC):
        rev = core >= 4
        q = core % 4
        bs = slice(q * BL, (q + 1) * BL)
        d = 1 if rev else 0
        W_ih = np.asarray(W_ih_b if rev else W_ih_f, np.float32)
        W_hh = np.asarray(W_hh_b if rev else W_hh_f, np.float32)
        b_i = np.asarray(b_ih_b if rev else b_ih_f, np.float32)
        b_h = np.asarray(b_hh_b if rev else b_hh_f, np.float32)
        in_maps.append(_prep_core_inputs(
            embeds[bs], h0[d, bs], c0[d, bs], W_ih, W_hh, b_i, b_h,
            np.asarray(W_lin, np.float32), rev))

    nc = _get_program()
    res = run_bass_kernel_spmd(nc, in_maps, list(range(NC)), trace=_trace)
    outs = res.results

    feats = np.zeros((B, S, T), np.float32)
    for q in range(4):
        bs = slice(q * BL, (q + 1) * BL)
        fwd = outs[q]["partial"].reshape(S, BL, T).transpose(1, 0, 2)
        bwd = outs[q + 4]["partial"].reshape(S, BL, T).transpose(1, 0, 2)
        feats[bs] = fwd + bwd[:, ::-1, :] + np.asarray(b_lin, np.float32)
    if _trace:
        return feats, res
    return feats



# revision 2
# speedup vs baseline: 5.1939x; 5.1939x over previous
"""BERT-LSTM-CRF kernel for Trainium2, 8 NeuronCores.

Wall-clock-optimized: under the axon tunnel the end-to-end time of a
warm kernel() call is dominated by host->device input bytes and
per-call compile/lowering overhead, not device compute. Design:

  * Batch-shard 8-way (8 samples/core); each core runs BOTH LSTM
    directions, so the aligned embeddings ship once (not once per
    direction-core as in the 4+4 direction split).
  * Ragged cut: the word aligner zero-pads past sent_len (<= 258 for
    this generator), so embeds rows t >= CUT(=272) are all-zero and
    xg degenerates to the bias row. Only [CUT*8, 768] embedding rows
    ship per core; the scans reuse the xg slice at t=CUT-1 (pure
    bias) for every step past the cut. Falls back to a full-length
    program if masks ever exceed the cut.
  * bf16 wire format for embeddings + weights (matmuls in bf16 with
    fp32 PSUM accumulation; cell state stays fp32).
  * Replicated weights (W_ih/W_hh/W_lin/bias, both directions) are
    sharded 1/8th per core on the host and AllGathered on device over
    NeuronLink, cutting their upload 8x.
  * fwd+bwd LSTM + output projection fused on device; output is the
    per-core feats [22, S*8] in bf16; host adds b_lin.
  * jax persistent compilation cache + memoized BIR serialization so
    warm calls skip the per-call NEFF recompile that otherwise costs
    seconds inside run_bass_kernel_spmd's fresh-jit path.

Per-call upload ~36MB vs ~305MB for the direction-split fp32 version.
"""
import os
import sys
import tempfile
import numpy as np

sys.path.insert(0, "/opt/trn_rl_repo")

B, S, D, H, T = 64, 512, 768, 384, 22
G4 = 4 * H            # 1536 gate rows
BL = 8                # batch per core
NC = 8
KD = D // 128         # 6 contraction chunks of the input GEMM
KH = H // 128         # 3 hidden chunks
CUT = 272             # compile-time ragged cut (>= max sent_len+1, x16)
STEPS = int(os.environ.get("KSTEPS", str(S)))
XCH = 4               # scan timesteps per xg DMA chunk

_cache = {}
_cfg_done = [False]

# gate-order permutation: torch [i,f,g,o] -> kernel [i,f,o,g]
_PERM = np.concatenate([np.arange(0, H), np.arange(H, 2 * H),
                        np.arange(3 * H, 4 * H), np.arange(2 * H, 3 * H)])


def _configure_jax_cache():
    """Persistent XLA compilation cache: the runner rebuilds its jit
    closure every call, so without this every warm call re-runs the
    multi-second NEFF compile."""
    if _cfg_done[0]:
        return
    _cfg_done[0] = True
    try:
        import jax
        jax.config.update(
            "jax_compilation_cache_dir",
            os.path.join(tempfile.gettempdir(), "jax_comp_cache"))
        jax.config.update("jax_persistent_cache_min_entry_size_bytes", -1)
        jax.config.update("jax_persistent_cache_min_compile_time_secs", 0.0)
    except Exception:
        pass


def _build_program(cut, steps, s_len=S):
    from concourse import bacc, tile, mybir
    from contextlib import ExitStack

    f32 = mybir.dt.float32
    bf16 = mybir.dt.bfloat16
    AF = mybir.ActivationFunctionType

    nc = bacc.Bacc("TRN2", target_bir_lowering=False, debug=False,
                   num_devices=NC)

    NR = (BL * cut) // 128   # emb row tiles

    emb = nc.dram_tensor("emb", [BL * cut, D], bf16, kind="ExternalInput")
    h0t = nc.dram_tensor("h0t", [2 * H, BL], bf16, kind="ExternalInput")
    c0 = nc.dram_tensor("c0", [BL, 2, H], f32, kind="ExternalInput")
    id128 = nc.dram_tensor("id128", [128, 128], bf16, kind="ExternalInput")
    id8 = nc.dram_tensor("id8", [8, 8], f32, kind="ExternalInput")
    # weight shards (1/8th each); full tensors assembled via AllGather
    wih_sh = nc.dram_tensor("wih_sh", [2 * D // NC, G4], bf16,
                            kind="ExternalInput")
    whh_sh = nc.dram_tensor("whh_sh", [2 * H // NC, G4], bf16,
                            kind="ExternalInput")
    wlin_sh = nc.dram_tensor("wlin_sh", [2 * H // NC, T], bf16,
                             kind="ExternalInput")
    bias_sh = nc.dram_tensor("bias_sh", [2 * G4 // NC], bf16,
                             kind="ExternalInput")
    wih_full = nc.dram_tensor("wih_full", [2 * D, G4], bf16,
                              addr_space="Shared")
    whh_full = nc.dram_tensor("whh_full", [2 * H, G4], bf16,
                              addr_space="Shared")
    wlin_full = nc.dram_tensor("wlin_full", [2 * H, T], bf16,
                               addr_space="Shared")
    bias_full = nc.dram_tensor("bias_full", [2, G4], bf16,
                               addr_space="Shared")
    out = nc.dram_tensor("out", [T, s_len * BL], bf16, kind="ExternalOutput")
    xg_dram = nc.dram_tensor("xg_scratch", [2, cut * BL, G4], f32)
    # collectives cannot read IO tensors: bounce shards through internal dram
    wih_bn = nc.dram_tensor("wih_bn", [2 * D // NC, G4], bf16)
    whh_bn = nc.dram_tensor("whh_bn", [2 * H // NC, G4], bf16)
    wlin_bn = nc.dram_tensor("wlin_bn", [2 * H // NC, T], bf16)
    bias_bn = nc.dram_tensor("bias_bn", [2 * G4 // NC], bf16)

    grp = [list(range(NC))]

    with tile.TileContext(nc) as tc, ExitStack() as big:
        for src, bn, full in ((wih_sh, wih_bn, wih_full),
                              (whh_sh, whh_bn, whh_full),
                              (wlin_sh, wlin_bn, wlin_full),
                              (bias_sh, bias_bn, bias_full)):
            nc.sync.dma_start(bn[:], src[:])
            nc.gpsimd.collective_compute(
                "AllGather", mybir.AluOpType.bypass, replica_groups=grp,
                ins=[bn[:]], outs=[full[:]])

        consts = big.enter_context(tc.tile_pool(name="consts", bufs=1))
        hist_pool = big.enter_context(tc.tile_pool(name="hist", bufs=1))

        id128_sb = consts.tile([128, 128], bf16, tag="id128")
        nc.sync.dma_start(id128_sb[:], id128[:])
        id8_sb = consts.tile([8, 8], f32, tag="id8")
        nc.sync.dma_start(id8_sb[:], id8[:])
        ones_sb = consts.tile([1, 128], bf16, tag="ones")
        nc.vector.memset(ones_sb[:], 1.0)

        whh_all = consts.tile([128, 2 * KH, G4], bf16, tag="whh")
        nc.sync.dma_start(whh_all[:],
                          whh_full.rearrange("(d k p) g -> p (d k) g",
                                             p=128, k=KH))
        wlin_all = consts.tile([128, 2 * KH, T], bf16, tag="wlin")
        nc.sync.dma_start(wlin_all[:],
                          wlin_full.rearrange("(d k p) t -> p (d k) t",
                                              p=128, k=KH))
        h0t_all = consts.tile([128, 2 * KH, BL], bf16, tag="h0t")
        nc.sync.dma_start(h0t_all[:],
                          h0t.rearrange("(d k p) b -> p (d k) b",
                                        p=128, k=KH))
        c0_sb = consts.tile([BL, 2, H], f32, tag="c0")
        nc.sync.dma_start(c0_sb[:], c0[:])
        bias_sb = [consts.tile([1, G4], bf16, tag=f"bias{d}",
                               name=f"bias{d}") for d in range(2)]
        for d in range(2):
            nc.sync.dma_start(bias_sb[d][:], bias_full[d:d + 1, :])
        # xg slice used for every step past the cut (== pure-bias row)
        bx_sb = [consts.tile([BL, G4], f32, tag=f"bx{d}", name=f"bx{d}")
                 for d in range(2)]

        # hidden history (transposed): hist[d][128, KH, S*BL], col = t*BL+b
        hist = []
        for d in range(2):
            ht = hist_pool.tile([128, KH, s_len * BL], bf16, tag=f"hist{d}",
                                name=f"hist{d}")
            hist.append(ht)
            nc.vector.memset(ht[:], 0.0)

        # ---------- phase 1: xg[d] = emb @ wih[d] + bias[d] ----------
        with ExitStack() as ph1:
            wp = ph1.enter_context(tc.tile_pool(name="wih", bufs=1))
            wih_all = wp.tile([128, 2 * KD, G4], bf16, tag="wih")
            nc.sync.dma_start(wih_all[:],
                              wih_full.rearrange("(d k p) g -> p (d k) g",
                                                 p=128, k=KD))
            ep = ph1.enter_context(tc.tile_pool(name="emb", bufs=3))
            etp = ph1.enter_context(tc.tile_pool(name="embT", bufs=3))
            xsp = ph1.enter_context(tc.tile_pool(name="xsb", bufs=3))
            pp = ph1.enter_context(
                tc.tile_pool(name="ph1ps", bufs=2, space="PSUM"))
            xp = ph1.enter_context(
                tc.tile_pool(name="xgps", bufs=3, space="PSUM"))
            for rt in range(NR):
                emb_sb = ep.tile([128, D], bf16, tag="emb")
                nc.sync.dma_start(emb_sb[:], emb[rt * 128:(rt + 1) * 128, :])
                etsb = etp.tile([128, D], bf16, tag="etsb")
                ps = pp.tile([128, D], bf16, tag="tps")
                for k in range(KD):
                    nc.tensor.transpose(ps[:, k * 128:(k + 1) * 128],
                                        emb_sb[:, k * 128:(k + 1) * 128],
                                        id128_sb[:])
                nc.vector.tensor_copy(etsb[:], ps[:])
                for d in range(2):
                    for n in range(3):
                        xps = xp.tile([128, 512], f32, tag="xps")
                        for k in range(KD):
                            nc.tensor.matmul(
                                xps[:],
                                etsb[:, k * 128:(k + 1) * 128],
                                wih_all[:, d * KD + k, n * 512:(n + 1) * 512],
                                start=(k == 0), stop=False)
                        nc.tensor.matmul(
                            xps[:], ones_sb[:],
                            bias_sb[d][:, n * 512:(n + 1) * 512],
                            start=False, stop=True)
                        xsb = xsp.tile([128, 512], f32, tag="xsb")
                        nc.vector.tensor_copy(xsb[:], xps[:])
                        nc.sync.dma_start(
                            xg_dram[d, rt * 128:(rt + 1) * 128,
                                    n * 512:(n + 1) * 512],
                            xsb[:])

        # bias-only xg rows for the tail steps
        for d in range(2):
            nc.sync.dma_start(
                bx_sb[d][:],
                xg_dram[d, (cut - 1) * BL:cut * BL, :])

        # ---------- phase 2: the two LSTM scans ----------
        with ExitStack() as ph2:
            xgp = ph2.enter_context(tc.tile_pool(name="xgin", bufs=2))
            gp = ph2.enter_context(
                tc.tile_pool(name="gps", bufs=2, space="PSUM"))
            htp = ph2.enter_context(
                tc.tile_pool(name="htps", bufs=2, space="PSUM"))
            sp = ph2.enter_context(tc.tile_pool(name="scan", bufs=2))
            cp = ph2.enter_context(tc.tile_pool(name="cbuf", bufs=2))

            for d in range(2):
                c_prev = c0_sb[:, d, :]
                xg_sb = None
                xg_lo = None
                for s in range(steps):
                    t_eff = s if d == 0 else s_len - 1 - s
                    if t_eff < cut:
                        # chunk-aligned DMA of XCH consecutive t slices
                        if d == 0:
                            lo = (s // XCH) * XCH
                            need = (s % XCH == 0)
                        else:
                            # t_eff descends cut-1 ... 0
                            j = cut - 1 - t_eff
                            lo = cut - 1 - (j // XCH) * XCH - (XCH - 1)
                            lo = max(lo, 0)
                            need = (j % XCH == 0)
                        if need:
                            nchunk = min(XCH, cut - lo)
                            xg_sb = xgp.tile([BL, XCH, G4], f32, tag="xg")
                            nc.sync.dma_start(
                                xg_sb[:, 0:nchunk, :],
                                xg_dram[d, lo * BL:(lo + nchunk) * BL, :]
                                .rearrange("(t b) g -> b t g", b=BL))
                            xg_lo = lo
                        xg_op = xg_sb[:, t_eff - xg_lo, :]
                    else:
                        xg_op = bx_sb[d][:]

                    if s == 0:
                        hT = [h0t_all[:, d * KH + k, :] for k in range(KH)]
                    else:
                        tp = t_eff - 1 if d == 0 else t_eff + 1
                        hT = [hist[d][:, k, tp * BL:(tp + 1) * BL]
                              for k in range(KH)]

                    g_ps = gp.tile([BL, G4], f32, tag="g")
                    for n in range(3):
                        for k in range(KH):
                            nc.tensor.matmul(
                                g_ps[:, n * 512:(n + 1) * 512],
                                hT[k],
                                whh_all[:, d * KH + k, n * 512:(n + 1) * 512],
                                start=(k == 0), stop=(k == KH - 1))
                    g_sb = sp.tile([BL, G4], f32, tag="gsb")
                    nc.vector.tensor_add(g_sb[:], g_ps[:], xg_op[:])
                    # gate order [i, f, o, g]
                    a_sb = sp.tile([BL, G4], f32, tag="asb")
                    nc.scalar.activation(a_sb[:, 0:3 * H], g_sb[:, 0:3 * H],
                                         AF.Sigmoid)
                    nc.scalar.activation(a_sb[:, 3 * H:G4], g_sb[:, 3 * H:G4],
                                         AF.Tanh)
                    c_new = cp.tile([BL, H], f32, tag="c")
                    tmp = sp.tile([BL, 2 * H], f32, tag="tmp")
                    nc.vector.tensor_mul(tmp[:, 0:H], a_sb[:, 0:H],
                                         a_sb[:, 3 * H:G4])      # i*tanh(g)
                    nc.vector.tensor_mul(c_new[:], a_sb[:, H:2 * H], c_prev)
                    nc.vector.tensor_add(c_new[:], c_new[:], tmp[:, 0:H])
                    nc.scalar.activation(tmp[:, H:2 * H], c_new[:], AF.Tanh)
                    h_sb = sp.tile([BL, H], f32, tag="h")
                    nc.vector.tensor_mul(h_sb[:], a_sb[:, 2 * H:3 * H],
                                         tmp[:, H:2 * H])
                    ht_ps = htp.tile([128, KH * BL], f32, tag="htps")
                    for k in range(KH):
                        nc.tensor.transpose(ht_ps[:, k * BL:(k + 1) * BL],
                                            h_sb[:, k * 128:(k + 1) * 128],
                                            id8_sb[:])
                    nc.vector.tensor_copy(
                        hist[d][:, :, t_eff * BL:(t_eff + 1) * BL],
                        ht_ps[:].rearrange("p (k b) -> p k b", k=KH))
                    c_prev = c_new[:]

        # ---------- phase 3: out = sum_d hist[d]^T @ wlin[d] ----------
        with ExitStack() as ph3:
            fp = ph3.enter_context(
                tc.tile_pool(name="fps", bufs=2, space="PSUM"))
            fsb = ph3.enter_context(tc.tile_pool(name="fsb", bufs=2))
            CW = 512 if (s_len * BL) % 512 == 0 else s_len * BL
            NCH = (s_len * BL) // CW
            for j in range(NCH):
                f_ps = fp.tile([T, CW], f32, tag="f")
                for d in range(2):
                    for k in range(KH):
                        nc.tensor.matmul(
                            f_ps[:],
                            wlin_all[:, d * KH + k, :],
                            hist[d][:, k, j * CW:(j + 1) * CW],
                            start=(d == 0 and k == 0),
                            stop=(d == 1 and k == KH - 1))
                f_sb = fsb.tile([T, CW], bf16, tag="fsb")
                nc.vector.tensor_copy(f_sb[:], f_ps[:])
                nc.sync.dma_start(out[:, j * CW:(j + 1) * CW], f_sb[:])

    nc.compile()
    # the PJRT lowering re-serializes the (immutable) module on every call
    # (~0.4s for this program); memoize it.
    _raw = [None]
    _orig = nc.to_json_bytes

    def _cached_json():
        if _raw[0] is None:
            _raw[0] = _orig()
        return _raw[0]

    nc.to_json_bytes = _cached_json
    return nc


def _get_program(cut, steps, s_len=S):
    key = (cut, steps, s_len)
    if key not in _cache:
        _cache[key] = _build_program(cut, steps, s_len)
    return _cache[key]


def _align_idx(start_ids, masks, cut):
    """Gather indices + keep mask for the first `cut` word slots."""
    sid = np.asarray(start_ids)
    msk = np.asarray(masks)
    Bb, Ss = sid.shape
    t = np.arange(cut)[None, :]
    n = (sid >= 0).sum(-1)
    last_sid = np.take_along_axis(sid, (n - 1)[:, None], axis=1)
    sid_c = sid[:, :cut]
    idx = np.where(t == 0, 0,
          np.where(t < n[:, None], sid_c - 1,
          np.where(t == n[:, None], last_sid, 0)))
    idx = np.clip(idx, 0, Ss - 1).astype(np.int64)
    sent_len = msk.sum(-1)
    keep = (t < sent_len[:, None])
    return idx, keep, int(sent_len.max())


def _host_prep(hidden_states, h0, c0, W_ih_f, W_hh_f, b_ih_f, b_hh_f,
               W_ih_b, W_hh_b, b_ih_b, b_hh_b, W_lin, b_lin,
               start_ids, masks, cut):
    import ml_dtypes
    bf16 = ml_dtypes.bfloat16

    hs = np.asarray(hidden_states, np.float32)
    idx, keep, _ = _align_idx(start_ids, masks, cut)
    gathered = np.take_along_axis(hs, idx[:, :, None], axis=1)  # [B,cut,D]
    gathered *= keep[:, :, None]
    emb_bf = gathered.astype(bf16)

    # replicated weights -> per-core 1/8 shards (views; concat copies later)
    def bfT(w, perm=None):
        w = np.asarray(w, np.float32).T
        if perm is not None:
            w = w[:, perm]
        return np.ascontiguousarray(w).astype(bf16)

    W_lin = np.asarray(W_lin, np.float32)
    wih_host = np.concatenate([bfT(W_ih_f, _PERM), bfT(W_ih_b, _PERM)], 0)
    whh_host = np.concatenate([bfT(W_hh_f, _PERM), bfT(W_hh_b, _PERM)], 0)
    wlin_host = np.concatenate(
        [np.ascontiguousarray(W_lin[:, :H].T).astype(bf16),
         np.ascontiguousarray(W_lin[:, H:].T).astype(bf16)], 0)
    bias_host = np.stack(
        [(np.asarray(b_ih_f, np.float32) + np.asarray(b_hh_f, np.float32))[_PERM],
         (np.asarray(b_ih_b, np.float32) + np.asarray(b_hh_b, np.float32))[_PERM]],
        0).astype(bf16)

    wih_shards = wih_host.reshape(NC, 2 * D // NC, G4)
    whh_shards = whh_host.reshape(NC, 2 * H // NC, G4)
    wlin_shards = wlin_host.reshape(NC, 2 * H // NC, T)
    bias_shards = bias_host.reshape(NC, 2 * G4 // NC)

    h0 = np.asarray(h0, np.float32)
    c0a = np.asarray(c0, np.float32)
    id128 = np.eye(128, dtype=bf16)
    id8 = np.eye(8, dtype=np.float32)

    in_maps = []
    for core in range(NC):
        bs = slice(core * BL, (core + 1) * BL)
        e = emb_bf[bs]                        # [BL, cut, D]
        e = np.ascontiguousarray(e.transpose(1, 0, 2)).reshape(-1, D)
        h0t = np.concatenate(
            [np.ascontiguousarray(h0[0, bs].T),
             np.ascontiguousarray(h0[1, bs].T)], 0).astype(bf16)  # [2H, BL]
        c0m = np.ascontiguousarray(
            np.stack([c0a[0, bs], c0a[1, bs]], 1))  # [BL, 2, H]
        in_maps.append({
            "emb": e,
            "h0t": h0t,
            "c0": c0m,
            "id128": id128,
            "id8": id8,
            "wih_sh": wih_shards[core],
            "whh_sh": whh_shards[core],
            "wlin_sh": wlin_shards[core],
            "bias_sh": bias_shards[core],
        })
    return in_maps


def kernel(hidden_states, h0, c0, W_ih_f, W_hh_f, b_ih_f, b_hh_f,
           W_ih_b, W_hh_b, b_ih_b, b_hh_b, W_lin, b_lin, start_ids, masks,
           _trace=False):
    _configure_jax_cache()
    from concourse.bass_utils import run_bass_kernel_spmd

    msk = np.asarray(masks)
    max_sent = int(msk.sum(-1).max())
    cut = CUT if max_sent <= CUT - 1 else S

    in_maps = _host_prep(
        hidden_states, h0, c0, W_ih_f, W_hh_f, b_ih_f, b_hh_f,
        W_ih_b, W_hh_b, b_ih_b, b_hh_b, W_lin, b_lin, start_ids, masks, cut)

    nc = _get_program(cut, STEPS)
    res = run_bass_kernel_spmd(nc, in_maps, list(range(NC)), trace=_trace)
    outs = res.results

    b_lin = np.asarray(b_lin, np.float32)
    feats = np.empty((B, S, T), np.float32)
    for core in range(NC):
        o = outs[core]["out"].astype(np.float32)       # [T, S*BL]
        o = o.reshape(T, S, BL).transpose(2, 1, 0)     # [BL, S, T]
        feats[core * BL:(core + 1) * BL] = o + b_lin
    if _trace:
        return feats, res
    return feats


# revision 4
# speedup vs baseline: 5.2040x; 1.0019x over previous
"""BERT-LSTM-CRF kernel for Trainium2, 8 NeuronCores.

Wall-clock-optimized: under the axon tunnel the end-to-end time of a
warm kernel() call is dominated by host->device input bytes and
per-call compile/lowering overhead, not device compute. Design:

  * Batch-shard 8-way (8 samples/core); each core runs BOTH LSTM
    directions, so the aligned embeddings ship once (not once per
    direction-core as in the 4+4 direction split).
  * Ragged cut: the word aligner zero-pads past sent_len (<= 258 for
    this generator), so embeds rows t >= CUT(=272) are all-zero and
    xg degenerates to the bias row. Only [CUT*8, 768] embedding rows
    ship per core; the scans reuse the xg slice at t=CUT-1 (pure
    bias) for every step past the cut. Falls back to a full-length
    program if masks ever exceed the cut.
  * bf16 wire format for embeddings + weights (matmuls in bf16 with
    fp32 PSUM accumulation; cell state stays fp32).
  * Replicated weights (W_ih/W_hh/W_lin/bias, both directions) are
    sharded 1/8th per core on the host and AllGathered on device over
    NeuronLink, cutting their upload 8x.
  * fwd+bwd LSTM + output projection fused on device; output is the
    per-core feats [22, S*8] in bf16; host adds b_lin.
  * jax persistent compilation cache + memoized BIR serialization so
    warm calls skip the per-call NEFF recompile that otherwise costs
    seconds inside run_bass_kernel_spmd's fresh-jit path.

Per-call upload ~36MB vs ~305MB for the direction-split fp32 version.
"""
import os
import sys
import tempfile
import numpy as np

sys.path.insert(0, "/opt/trn_rl_repo")

B, S, D, H, T = 64, 512, 768, 384, 22
G4 = 4 * H            # 1536 gate rows
BL = 8                # batch per core
NC = 8
KD = D // 128         # 6 contraction chunks of the input GEMM
KH = H // 128         # 3 hidden chunks
CUT = 272             # compile-time ragged cut (>= max sent_len+1, x16)
STEPS = int(os.environ.get("KSTEPS", str(S)))
XCH = 4               # scan timesteps per xg DMA chunk

_cache = {}
_cfg_done = [False]

# gate-order permutation: torch [i,f,g,o] -> kernel [i,f,o,g]
_PERM = np.concatenate([np.arange(0, H), np.arange(H, 2 * H),
                        np.arange(3 * H, 4 * H), np.arange(2 * H, 3 * H)])


def _configure_jax_cache():
    """Persistent XLA compilation cache: the runner rebuilds its jit
    closure every call, so without this every warm call re-runs the
    multi-second NEFF compile."""
    if _cfg_done[0]:
        return
    _cfg_done[0] = True
    try:
        import jax
        jax.config.update(
            "jax_compilation_cache_dir",
            os.path.join(tempfile.gettempdir(), "jax_comp_cache"))
        jax.config.update("jax_persistent_cache_min_entry_size_bytes", -1)
        jax.config.update("jax_persistent_cache_min_compile_time_secs", 0.0)
    except Exception:
        pass


def _build_program(cut, steps, s_len=S):
    from concourse import bacc, tile, mybir
    from contextlib import ExitStack

    f32 = mybir.dt.float32
    bf16 = mybir.dt.bfloat16
    AF = mybir.ActivationFunctionType

    nc = bacc.Bacc("TRN2", target_bir_lowering=False, debug=False,
                   num_devices=NC)

    NR = (BL * cut) // 128   # emb row tiles

    emb = nc.dram_tensor("emb", [BL * cut, D], bf16, kind="ExternalInput")
    h0t = nc.dram_tensor("h0t", [2 * H, BL], bf16, kind="ExternalInput")
    c0 = nc.dram_tensor("c0", [BL, 2, H], f32, kind="ExternalInput")
    id128 = nc.dram_tensor("id128", [128, 128], bf16, kind="ExternalInput")
    id8 = nc.dram_tensor("id8", [8, 8], f32, kind="ExternalInput")
    # weight shards (1/8th each); full tensors assembled via AllGather
    wih_sh = nc.dram_tensor("wih_sh", [2 * D // NC, G4], bf16,
                            kind="ExternalInput")
    whh_sh = nc.dram_tensor("whh_sh", [2 * H // NC, G4], bf16,
                            kind="ExternalInput")
    wlin_sh = nc.dram_tensor("wlin_sh", [2 * H // NC, T], bf16,
                             kind="ExternalInput")
    bias_sh = nc.dram_tensor("bias_sh", [2 * G4 // NC], bf16,
                             kind="ExternalInput")
    wih_full = nc.dram_tensor("wih_full", [2 * D, G4], bf16,
                              addr_space="Shared")
    whh_full = nc.dram_tensor("whh_full", [2 * H, G4], bf16,
                              addr_space="Shared")
    wlin_full = nc.dram_tensor("wlin_full", [2 * H, T], bf16,
                               addr_space="Shared")
    bias_full = nc.dram_tensor("bias_full", [2, G4], bf16,
                               addr_space="Shared")
    out = nc.dram_tensor("out", [T, s_len * BL], bf16, kind="ExternalOutput")
    xg_dram = nc.dram_tensor("xg_scratch", [2, cut * BL, G4], f32)
    # collectives cannot read IO tensors: bounce shards through internal dram
    wih_bn = nc.dram_tensor("wih_bn", [2 * D // NC, G4], bf16)
    whh_bn = nc.dram_tensor("whh_bn", [2 * H // NC, G4], bf16)
    wlin_bn = nc.dram_tensor("wlin_bn", [2 * H // NC, T], bf16)
    bias_bn = nc.dram_tensor("bias_bn", [2 * G4 // NC], bf16)

    grp = [list(range(NC))]

    with tile.TileContext(nc) as tc, ExitStack() as big:
        for src, bn, full in ((wih_sh, wih_bn, wih_full),
                              (whh_sh, whh_bn, whh_full),
                              (wlin_sh, wlin_bn, wlin_full),
                              (bias_sh, bias_bn, bias_full)):
            nc.sync.dma_start(bn[:], src[:])
            nc.gpsimd.collective_compute(
                "AllGather", mybir.AluOpType.bypass, replica_groups=grp,
                ins=[bn[:]], outs=[full[:]])

        consts = big.enter_context(tc.tile_pool(name="consts", bufs=1))
        hist_pool = big.enter_context(tc.tile_pool(name="hist", bufs=1))

        id128_sb = consts.tile([128, 128], bf16, tag="id128")
        nc.sync.dma_start(id128_sb[:], id128[:])
        id8_sb = consts.tile([8, 8], f32, tag="id8")
        nc.sync.dma_start(id8_sb[:], id8[:])
        ones_sb = consts.tile([1, 128], bf16, tag="ones")
        nc.vector.memset(ones_sb[:], 1.0)

        whh_all = consts.tile([128, 2 * KH, G4], bf16, tag="whh")
        nc.sync.dma_start(whh_all[:],
                          whh_full.rearrange("(d k p) g -> p (d k) g",
                                             p=128, k=KH))
        wlin_all = consts.tile([128, 2 * KH, T], bf16, tag="wlin")
        nc.sync.dma_start(wlin_all[:],
                          wlin_full.rearrange("(d k p) t -> p (d k) t",
                                              p=128, k=KH))
        h0t_all = consts.tile([128, 2 * KH, BL], bf16, tag="h0t")
        nc.sync.dma_start(h0t_all[:],
                          h0t.rearrange("(d k p) b -> p (d k) b",
                                        p=128, k=KH))
        c0_sb = consts.tile([BL, 2, H], f32, tag="c0")
        nc.sync.dma_start(c0_sb[:], c0[:])
        bias_sb = [consts.tile([1, G4], bf16, tag=f"bias{d}",
                               name=f"bias{d}") for d in range(2)]
        for d in range(2):
            nc.sync.dma_start(bias_sb[d][:], bias_full[d:d + 1, :])
        # xg slice used for every step past the cut (== pure-bias row)
        bx_sb = [consts.tile([BL, G4], f32, tag=f"bx{d}", name=f"bx{d}")
                 for d in range(2)]

        # hidden history (transposed): hist[d][128, KH, S*BL], col = t*BL+b
        hist = []
        for d in range(2):
            ht = hist_pool.tile([128, KH, s_len * BL], bf16, tag=f"hist{d}",
                                name=f"hist{d}")
            hist.append(ht)
            nc.vector.memset(ht[:], 0.0)

        # ---------- phase 1: xg[d] = emb @ wih[d] + bias[d] ----------
        with ExitStack() as ph1:
            wp = ph1.enter_context(tc.tile_pool(name="wih", bufs=1))
            wih_all = wp.tile([128, 2 * KD, G4], bf16, tag="wih")
            nc.sync.dma_start(wih_all[:],
                              wih_full.rearrange("(d k p) g -> p (d k) g",
                                                 p=128, k=KD))
            ep = ph1.enter_context(tc.tile_pool(name="emb", bufs=3))
            etp = ph1.enter_context(tc.tile_pool(name="embT", bufs=3))
            xsp = ph1.enter_context(tc.tile_pool(name="xsb", bufs=3))
            pp = ph1.enter_context(
                tc.tile_pool(name="ph1ps", bufs=2, space="PSUM"))
            xp = ph1.enter_context(
                tc.tile_pool(name="xgps", bufs=3, space="PSUM"))
            for rt in range(NR):
                emb_sb = ep.tile([128, D], bf16, tag="emb")
                nc.sync.dma_start(emb_sb[:], emb[rt * 128:(rt + 1) * 128, :])
                etsb = etp.tile([128, D], bf16, tag="etsb")
                ps = pp.tile([128, D], bf16, tag="tps")
                for k in range(KD):
                    nc.tensor.transpose(ps[:, k * 128:(k + 1) * 128],
                                        emb_sb[:, k * 128:(k + 1) * 128],
                                        id128_sb[:])
                nc.vector.tensor_copy(etsb[:], ps[:])
                for d in range(2):
                    for n in range(3):
                        xps = xp.tile([128, 512], f32, tag="xps")
                        for k in range(KD):
                            nc.tensor.matmul(
                                xps[:],
                                etsb[:, k * 128:(k + 1) * 128],
                                wih_all[:, d * KD + k, n * 512:(n + 1) * 512],
                                start=(k == 0), stop=False)
                        nc.tensor.matmul(
                            xps[:], ones_sb[:],
                            bias_sb[d][:, n * 512:(n + 1) * 512],
                            start=False, stop=True)
                        xsb = xsp.tile([128, 512], f32, tag="xsb")
                        nc.vector.tensor_copy(xsb[:], xps[:])
                        nc.sync.dma_start(
                            xg_dram[d, rt * 128:(rt + 1) * 128,
                                    n * 512:(n + 1) * 512],
                            xsb[:])

        # bias-only xg rows for the tail steps
        for d in range(2):
            nc.sync.dma_start(
                bx_sb[d][:],
                xg_dram[d, (cut - 1) * BL:cut * BL, :])

        # ---------- phase 2: the two LSTM scans ----------
        with ExitStack() as ph2:
            xgp = ph2.enter_context(tc.tile_pool(name="xgin", bufs=2))
            gp = ph2.enter_context(
                tc.tile_pool(name="gps", bufs=2, space="PSUM"))
            htp = ph2.enter_context(
                tc.tile_pool(name="htps", bufs=2, space="PSUM"))
            sp = ph2.enter_context(tc.tile_pool(name="scan", bufs=2))
            cp = ph2.enter_context(tc.tile_pool(name="cbuf", bufs=2))

            for d in range(2):
                c_prev = c0_sb[:, d, :]
                xg_sb = None
                xg_lo = None
                for s in range(steps):
                    t_eff = s if d == 0 else s_len - 1 - s
                    if t_eff < cut:
                        # chunk-aligned DMA of XCH consecutive t slices
                        if d == 0:
                            lo = (s // XCH) * XCH
                            need = (s % XCH == 0)
                        else:
                            # t_eff descends cut-1 ... 0
                            j = cut - 1 - t_eff
                            lo = cut - 1 - (j // XCH) * XCH - (XCH - 1)
                            lo = max(lo, 0)
                            need = (j % XCH == 0)
                        if need:
                            nchunk = min(XCH, cut - lo)
                            xg_sb = xgp.tile([BL, XCH, G4], f32, tag="xg")
                            nc.sync.dma_start(
                                xg_sb[:, 0:nchunk, :],
                                xg_dram[d, lo * BL:(lo + nchunk) * BL, :]
                                .rearrange("(t b) g -> b t g", b=BL))
                            xg_lo = lo
                        xg_op = xg_sb[:, t_eff - xg_lo, :]
                    else:
                        xg_op = bx_sb[d][:]

                    if s == 0:
                        hT = [h0t_all[:, d * KH + k, :] for k in range(KH)]
                    else:
                        tp = t_eff - 1 if d == 0 else t_eff + 1
                        hT = [hist[d][:, k, tp * BL:(tp + 1) * BL]
                              for k in range(KH)]

                    g_ps = gp.tile([BL, G4], f32, tag="g")
                    for n in range(3):
                        for k in range(KH):
                            nc.tensor.matmul(
                                g_ps[:, n * 512:(n + 1) * 512],
                                hT[k],
                                whh_all[:, d * KH + k, n * 512:(n + 1) * 512],
                                start=(k == 0), stop=(k == KH - 1))
                    g_sb = sp.tile([BL, G4], f32, tag="gsb")
                    nc.vector.tensor_add(g_sb[:], g_ps[:], xg_op[:])
                    # gate order [i, f, o, g]
                    a_sb = sp.tile([BL, G4], f32, tag="asb")
                    nc.scalar.activation(a_sb[:, 0:3 * H], g_sb[:, 0:3 * H],
                                         AF.Sigmoid)
                    nc.scalar.activation(a_sb[:, 3 * H:G4], g_sb[:, 3 * H:G4],
                                         AF.Tanh)
                    c_new = cp.tile([BL, H], f32, tag="c")
                    tmp = sp.tile([BL, 2 * H], f32, tag="tmp")
                    nc.vector.tensor_mul(tmp[:, 0:H], a_sb[:, 0:H],
                                         a_sb[:, 3 * H:G4])      # i*tanh(g)
                    nc.vector.tensor_mul(c_new[:], a_sb[:, H:2 * H], c_prev)
                    nc.vector.tensor_add(c_new[:], c_new[:], tmp[:, 0:H])
                    nc.scalar.activation(tmp[:, H:2 * H], c_new[:], AF.Tanh)
                    h_sb = sp.tile([BL, H], f32, tag="h")
                    nc.vector.tensor_mul(h_sb[:], a_sb[:, 2 * H:3 * H],
                                         tmp[:, H:2 * H])
                    ht_ps = htp.tile([128, KH * BL], f32, tag="htps")
                    for k in range(KH):
                        nc.tensor.transpose(ht_ps[:, k * BL:(k + 1) * BL],
                                            h_sb[:, k * 128:(k + 1) * 128],
                                            id8_sb[:])
                    nc.vector.tensor_copy(
                        hist[d][:, :, t_eff * BL:(t_eff + 1) * BL],
                        ht_ps[:].rearrange("p (k b) -> p k b", k=KH))
                    c_prev = c_new[:]

        # ---------- phase 3: out = sum_d hist[d]^T @ wlin[d] ----------
        with ExitStack() as ph3:
            fp = ph3.enter_context(
                tc.tile_pool(name="fps", bufs=2, space="PSUM"))
            fsb = ph3.enter_context(tc.tile_pool(name="fsb", bufs=2))
            CW = 512 if (s_len * BL) % 512 == 0 else s_len * BL
            NCH = (s_len * BL) // CW
            for j in range(NCH):
                f_ps = fp.tile([T, CW], f32, tag="f")
                for d in range(2):
                    for k in range(KH):
                        nc.tensor.matmul(
                            f_ps[:],
                            wlin_all[:, d * KH + k, :],
                            hist[d][:, k, j * CW:(j + 1) * CW],
                            start=(d == 0 and k == 0),
                            stop=(d == 1 and k == KH - 1))
                f_sb = fsb.tile([T, CW], bf16, tag="fsb")
                nc.vector.tensor_copy(f_sb[:], f_ps[:])
                nc.sync.dma_start(out[:, j * CW:(j + 1) * CW], f_sb[:])

    nc.compile()
    # the PJRT lowering re-serializes the (immutable) module on every call
    # (~0.4s for this program); memoize it.
    _raw = [None]
    _orig = nc.to_json_bytes

    def _cached_json():
        if _raw[0] is None:
            _raw[0] = _orig()
        return _raw[0]

    nc.to_json_bytes = _cached_json
    return nc


def _get_program(cut, steps, s_len=S):
    key = (cut, steps, s_len)
    if key not in _cache:
        _cache[key] = _build_program(cut, steps, s_len)
    return _cache[key]


def _align_idx(start_ids, masks, cut):
    """Gather indices + keep mask for the first `cut` word slots."""
    sid = np.asarray(start_ids)
    msk = np.asarray(masks)
    Bb, Ss = sid.shape
    t = np.arange(cut)[None, :]
    n = (sid >= 0).sum(-1)
    last_sid = np.take_along_axis(sid, (n - 1)[:, None], axis=1)
    sid_c = sid[:, :cut]
    idx = np.where(t == 0, 0,
          np.where(t < n[:, None], sid_c - 1,
          np.where(t == n[:, None], last_sid, 0)))
    idx = np.clip(idx, 0, Ss - 1).astype(np.int64)
    sent_len = msk.sum(-1)
    keep = (t < sent_len[:, None])
    return idx, keep, int(sent_len.max())


def _host_prep(hidden_states, h0, c0, W_ih_f, W_hh_f, b_ih_f, b_hh_f,
               W_ih_b, W_hh_b, b_ih_b, b_hh_b, W_lin, b_lin,
               start_ids, masks, cut):
    import ml_dtypes
    bf16 = ml_dtypes.bfloat16

    hs = np.asarray(hidden_states, np.float32)
    idx, keep, _ = _align_idx(start_ids, masks, cut)
    gathered = np.take_along_axis(hs, idx[:, :, None], axis=1)  # [B,cut,D]
    gathered *= keep[:, :, None]
    emb_bf = gathered.astype(bf16)

    # replicated weights -> per-core 1/8 shards (views; concat copies later)
    def bfT(w, perm=None):
        w = np.asarray(w, np.float32).T
        if perm is not None:
            w = w[:, perm]
        return np.ascontiguousarray(w).astype(bf16)

    W_lin = np.asarray(W_lin, np.float32)
    wih_host = np.concatenate([bfT(W_ih_f, _PERM), bfT(W_ih_b, _PERM)], 0)
    whh_host = np.concatenate([bfT(W_hh_f, _PERM), bfT(W_hh_b, _PERM)], 0)
    wlin_host = np.concatenate(
        [np.ascontiguousarray(W_lin[:, :H].T).astype(bf16),
         np.ascontiguousarray(W_lin[:, H:].T).astype(bf16)], 0)
    bias_host = np.stack(
        [(np.asarray(b_ih_f, np.float32) + np.asarray(b_hh_f, np.float32))[_PERM],
         (np.asarray(b_ih_b, np.float32) + np.asarray(b_hh_b, np.float32))[_PERM]],
        0).astype(bf16)

    wih_shards = wih_host.reshape(NC, 2 * D // NC, G4)
    whh_shards = whh_host.reshape(NC, 2 * H // NC, G4)
    wlin_shards = wlin_host.reshape(NC, 2 * H // NC, T)
    bias_shards = bias_host.reshape(NC, 2 * G4 // NC)

    h0 = np.asarray(h0, np.float32)
    c0a = np.asarray(c0, np.float32)
    id128 = np.eye(128, dtype=bf16)
    id8 = np.eye(8, dtype=np.float32)

    in_maps = []
    for core in range(NC):
        bs = slice(core * BL, (core + 1) * BL)
        e = emb_bf[bs]                        # [BL, cut, D]
        e = np.ascontiguousarray(e.transpose(1, 0, 2)).reshape(-1, D)
        h0t = np.concatenate(
            [np.ascontiguousarray(h0[0, bs].T),
             np.ascontiguousarray(h0[1, bs].T)], 0).astype(bf16)  # [2H, BL]
        c0m = np.ascontiguousarray(
            np.stack([c0a[0, bs], c0a[1, bs]], 1))  # [BL, 2, H]
        in_maps.append({
            "emb": e,
            "h0t": h0t,
            "c0": c0m,
            "id128": id128,
            "id8": id8,
            "wih_sh": wih_shards[core],
            "whh_sh": whh_shards[core],
            "wlin_sh": wlin_shards[core],
            "bias_sh": bias_shards[core],
        })
    return in_maps


def kernel(hidden_states, h0, c0, W_ih_f, W_hh_f, b_ih_f, b_hh_f,
           W_ih_b, W_hh_b, b_ih_b, b_hh_b, W_lin, b_lin, start_ids, masks,
           _trace=False):
    _configure_jax_cache()
    from concourse.bass_utils import run_bass_kernel_spmd

    msk = np.asarray(masks)
    max_sent = int(msk.sum(-1).max())
    cut = CUT if max_sent <= CUT - 1 else S

    in_maps = _host_prep(
        hidden_states, h0, c0, W_ih_f, W_hh_f, b_ih_f, b_hh_f,
        W_ih_b, W_hh_b, b_ih_b, b_hh_b, W_lin, b_lin, start_ids, masks, cut)

    nc = _get_program(cut, STEPS)
    res = run_bass_kernel_spmd(nc, in_maps, list(range(NC)), trace=_trace)
    outs = res.results

    b_lin = np.asarray(b_lin, np.float32)
    feats = np.empty((B, S, T), np.float32)
    for core in range(NC):
        o = outs[core]["out"].astype(np.float32)       # [T, S*BL]
        o = o.reshape(T, S, BL).transpose(2, 1, 0)     # [BL, S, T]
        feats[core * BL:(core + 1) * BL] = o + b_lin
    if _trace:
        return feats, res
    return feats
